# revision 2
# baseline (speedup 1.0000x reference)
"""LIF spike scan kernel for Trainium2, SPMD over 8 NeuronCores.

Problem: x [B=64, T=8, C=128, H=32, W=32] f32.  Per (b,c,h,w) pixel, scan
over T:  v = tau*u + x_t ; s_t = (v > 1) ; u = v*(v <= 1).  Output spikes
[B, T, C, H, W] f32.

v2 design (vs v1): the scan is a pure-DVE sequential chain over two
"super-groups" (SG) of 4 batch rows each, with all element-wise ops fused
to FD=4096 (4 rows x 1024 pixels) to amortize DVE per-op overhead:
    g = (v <= 4096) * 0.5     tensor_scalar  i16 -> f16 {0,0.5}   4x mode
    m = v * g                 tensor_tensor  i16 x f16 -> i16     2x_1P
    v' = m + q_t              tensor_tensor  i16 + i16 (in place) 2x_1P
The spikes for all 8 steps of a group are bit-packed into ONE byte by the
PE: pack += 2^(tl+1) * g accumulated in PSUM f32 across the full scan
(byte = sum 2^tl * keep in [0,255]), then ACT copies to u8 and a small DMA
ships it: output traffic is x256 smaller than f32 spikes.  Host ships
q = round(x * 2^12) int16 (threshold 4096); i16 writeback rounds to
nearest even, numerics identical to v1 (rel err 1.54e-2 vs 2e-2 gate).

Sharding: pure batch-parallel across 8 cores, no collectives.
"""

import numpy as np

B, T, C, HW = 64, 8, 128, 32 * 32
N_CORES = 8
B_LOC = B // N_CORES          # 8 batch rows per core
SCALE = 2.0 ** -12
THI = 4096.0                  # threshold in scaled domain
NSG = 2                       # super-groups per core
SGB = B_LOC // NSG            # batch rows per super-group (4)
FP = SGB * HW                 # fused free dim (4096)
FG = FP // 2                  # pack free dim per group (2048)

_cache = {}


def _build_nc():
    from concourse import bacc, mybir, tile

    op = mybir.AluOpType
    nc = bacc.Bacc(
        "TRN2", target_bir_lowering=False, debug=False, num_devices=N_CORES
    )
    i16, f16, f32 = mybir.dt.int16, mybir.dt.float16, mybir.dt.float32
    u8 = mybir.dt.uint8
    # q pre-shuffled on host to [sg*T + t, c, (g bl hw)]
    x_ext = nc.dram_tensor(
        "x", [NSG * T, C, FP], i16, kind="ExternalInput"
    ).ap()
    # Pack weights: w[:, tl*C:(tl+1)*C] = 2^(tl+1) * I  (f16, exact)
    w_ext = nc.dram_tensor("w", [C, T * C], f16, kind="ExternalInput").ap()
    # One byte-plane per group: bit tl = keep at step tl.
    out_ext = nc.dram_tensor(
        "out", [NSG * 2, C, FG], u8, kind="ExternalOutput"
    ).ap()

    with tile.TileContext(nc) as tc:
        with tc.tile_pool(name="pool", bufs=2) as pool, tc.tile_pool(
            name="psum", bufs=2, space="PSUM"
        ) as ppool:
            wt = pool.tile([C, T * C], f16, tag="w", bufs=1)
            # weights on the scalar queue: sync queue starts x chunk 0 at t=0
            nc.scalar.dma_start(out=wt, in_=w_ext)
            xc = {}
            for sg in range(NSG):
                for t in range(T):
                    xc[t] = pool.tile(
                        [C, FP], i16, tag="x", bufs=5, name=f"x{sg}_{t}"
                    )
                    nc.sync.dma_start(out=xc[t], in_=x_ext[sg * T + t])
                pk = [
                    ppool.tile([C, FG], f32, tag="pk", name=f"pk{sg}_{g}")
                    for g in range(2)
                ]
                for t in range(T):
                    if t > 0:
                        # v = m + q_t (in place over the x slot)
                        nc.vector.tensor_tensor(
                            out=xc[t], in0=mt, in1=xc[t], op=op.add
                        )
                    gt = pool.tile([C, FP], f16, tag="g", bufs=4,
                                   name=f"g{sg}_{t}")
                    # keep-gate with tau folded in: {0, 0.5} f16 (4x)
                    nc.vector.tensor_scalar(
                        out=gt, in0=xc[t], scalar1=THI, scalar2=0.5,
                        op0=op.is_le, op1=op.mult,
                    )
                    if t < T - 1:
                        mt = pool.tile([C, FP], i16, tag="m", bufs=2,
                                       name=f"m{sg}_{t}")
                        # m = v * g  (reset + tau; i16 x f16, 2x_1P)
                        nc.vector.tensor_tensor(
                            out=mt, in0=xc[t], in1=gt, op=op.mult
                        )
                    # pack += 2^(tl+1) * g  (PE, f32 PSUM, exact)
                    for g in range(2):
                        for j in range(0, FG, 512):
                            nc.tensor.matmul(
                                pk[g][:, j : j + 512],
                                wt[:, t * C : (t + 1) * C],
                                gt[:, g * FG + j : g * FG + j + 512],
                                start=(t == 0),
                                stop=(t == T - 1),
                            )
                for g in range(2):
                    pu = pool.tile([C, FG], u8, tag="pu", bufs=4,
                                   name=f"pu{sg}_{g}")
                    nc.scalar.copy(out=pu, in_=pk[g])
                    nc.scalar.dma_start(out=out_ext[sg * 2 + g], in_=pu)
    nc.compile()
    return nc


def _run(x: np.ndarray, trace: bool = False, tmpdir=None):
    from concourse.bass_utils import run_bass_kernel_spmd

    if "nc" not in _cache:
        _cache["nc"] = _build_nc()
    nc = _cache["nc"]
    x = np.asarray(x)
    q = np.clip(np.rint(x * np.float32(1.0 / SCALE)), -32768, 32767).astype(
        np.int16
    )
    # q[b=(sg*4+g*2+bl), t, c, hw] -> [core, (sg t), c, (g bl hw)]
    q7 = q.reshape(N_CORES, NSG, 2, 2, T, C, HW)
    q_shuf = np.ascontiguousarray(q7.transpose(0, 1, 4, 5, 2, 3, 6)).reshape(
        N_CORES, NSG * T, C, FP
    )
    w = np.zeros((C, T * C), dtype=np.float16)
    for t in range(T):
        w[np.arange(C), t * C + np.arange(C)] = np.float16(2.0 ** (t + 1))
    in_maps = [{"x": q_shuf[i], "w": w} for i in range(N_CORES)]
    res = run_bass_kernel_spmd(
        nc, in_maps, core_ids=list(range(N_CORES)), trace=trace, tmpdir=tmpdir
    )
    _cache["last_results"] = res
    outs = [res.results[i]["out"] for i in range(N_CORES)]
    # bytes [core, (sg g), c, (bl hw)]; bit tl = keep at t=tl
    by = np.stack(outs, axis=0).reshape(N_CORES, NSG, 2, 1, C, 2, HW)
    by = by.astype(np.uint8)
    tl = np.arange(T, dtype=np.uint8).reshape(1, 1, 1, T, 1, 1, 1)
    keep = (by >> tl) & np.uint8(1)        # [core, sg, g, t, c, bl, hw]
    spk = (1 - keep).astype(np.float32)
    out = spk.transpose(0, 1, 2, 5, 3, 4, 6).reshape(B, T, C, HW)
    return np.ascontiguousarray(out).reshape(B, T, C, 32, 32)


def kernel(x: np.ndarray) -> np.ndarray:
    return _run(x, trace=False)
